# revision 10
# baseline (speedup 1.0000x reference)
"""GCN residual block on 8 Trainium2 NeuronCores.

y = relu(gcn_conv(x)) -> relu(@W_lin + b_lin) -> + x

Strategy (memory-bound regime):
  - Nodes assigned to 8 cores x 98 groups of 128 by round-robin dealing in
    descending in-degree order, which balances edge counts per (group,
    bucket) cell across cores (the SPMD program sizes every cell at the
    max over cores, so balance directly cuts gather padding).
  - Real edges partitioned by dst core, grouped by dst group, bucketed by
    src window (6 windows of 16768 rows so indices fit int16). Gathered
    x[src] rows (bf16, 256B) via gpsimd dma_gather per (span-of-groups,
    bucket), spread across 4 SWDGE queues for concurrent descriptor
    drain (the gather is per-descriptor-rate-bound, not bandwidth-bound).
  - Scatter-add becomes PE matmuls: per 128-edge tile build a selection
    matrix S[e, d] = (iota[d] == dst_slot[e]) * norm[e] with one DVE
    tensor_scalar, then aggT += G_t^T @ S_t accumulated in PSUM per group.
  - Self-loops never touch DMA: their contribution is one fp32 matmul per
    group, aggT += xo_g^T @ D_g with D_g = diag(1/deg) built on DVE.
  - Per-group fp32 chain in transposed orientation: W^T @ aggT -> relu+bias
    (bias is per-partition there) -> W_lin^T @ . -> relu+bias -> PE
    transpose -> + x residual -> DMA out. Host unpermutes rows at the end.
"""

import sys

sys.path.insert(0, "/opt/trn_rl_repo")

import numpy as np
import ml_dtypes
from contextlib import ExitStack

import concourse.bass as bass
import concourse.mybir as mybir
import concourse.tile as tile
from concourse import bacc
from concourse.bass_utils import run_bass_kernel_spmd

N_NODES = 100000
N_EDGES = 1600000
H = 128
NCORES = 8
P = 128
NG = 98  # groups per core
NGRP = NCORES * NG  # 784 global groups
NPAD = NG * P  # padded nodes per core = 12544
NBUK = 6
WIN = 16768  # src window per bucket (int16-addressable)
SPAN = 8  # groups per gather call batch

F32 = mybir.dt.float32
BF16 = mybir.dt.bfloat16
I16 = mybir.dt.int16

TRACE = False  # set True (e.g. from test.py) to capture an NTFF profile
LAST_RESULT = None
LAST_NC = None
LAST_IN_MAPS = None
GATHER_ONLY = False  # debug: skip compute, only gathers
COMPUTE_ONLY = False  # debug: skip gathers, compute on stale SBUF
REPEAT = 1  # debug: repeat the whole body R times for overhead-free timing
NQUEUES = 4  # SWDGE descriptor queues; bucket b uses queue b % NQUEUES
PSUM_BUFS = 2  # buffers for each PSUM pool (agg, chain)
SW_BUFS = 6  # buffers for the DVE selection-matrix pool


def _preprocess(x, edge_index):
    """Host-side graph prep. Returns per-core SBUF-layout arrays plus the
    shared static layout (tiles per (group, bucket)) and the node
    permutation used for sharding."""
    src = np.ascontiguousarray(edge_index[0]).astype(np.int64)
    dst = np.ascontiguousarray(edge_index[1]).astype(np.int64)

    indeg = np.bincount(dst, minlength=N_NODES)
    deg = (indeg + 1).astype(np.float64)  # + self-loop
    dinv = 1.0 / np.sqrt(deg)
    norm = (dinv[src] * dinv[dst]).astype(np.float32)
    selfw = (dinv * dinv).astype(np.float32)

    # balanced node -> (core, group, slot): deal in descending degree order
    order = np.argsort(-indeg, kind="stable")
    ggrp = np.empty(N_NODES, dtype=np.int64)
    slot = np.empty(N_NODES, dtype=np.int64)
    j = np.arange(N_NODES)
    ggrp[order] = j % NGRP
    slot[order] = j // NGRP
    core_of = ggrp // NG
    g_of = ggrp % NG

    # node_of[c, g*P + p] = global node id or -1
    node_of = np.full((NCORES, NPAD), -1, dtype=np.int64)
    node_of[core_of, g_of * P + slot] = np.arange(N_NODES)

    dc = core_of[dst]
    dg = g_of[dst]
    dp = slot[dst]
    b = src // WIN

    per_core = []
    counts = np.zeros((NCORES, NG, NBUK), dtype=np.int64)
    for c in range(NCORES):
        m = dc == c
        s_c = src[m]
        p_c = dp[m]
        g_c = dg[m]
        w_c = norm[m]
        b_c = b[m]
        key = (g_c * NBUK + b_c).astype(np.int64)
        o = np.argsort(key, kind="stable")
        s_c, p_c, w_c, key = s_c[o], p_c[o], w_c[o], key[o]
        counts[c] = np.bincount(key, minlength=NG * NBUK).reshape(NG, NBUK)
        per_core.append((s_c, p_c, w_c, key))

    # shared across cores: tiles per (group, bucket)
    tiles_gb = (counts.max(axis=0) + P - 1) // P  # [NG, NBUK]
    tiles_gb = np.maximum(tiles_gb, 1)
    NT = int(tiles_gb.sum())  # total tile columns per core

    # per-bucket edge-slot layout offsets (tiles), ordered by group
    buk_tile_off = np.zeros((NG, NBUK), dtype=np.int64)
    for bb in range(NBUK):
        buk_tile_off[:, bb] = np.concatenate([[0], np.cumsum(tiles_gb[:-1, bb])])
    buk_len = tiles_gb.sum(axis=0) * P  # slots per bucket stream

    # processing order (meta columns): for g, for b, for t
    proc_off = np.zeros((NG, NBUK), dtype=np.int64)
    flat = tiles_gb.reshape(-1)
    proc_off.reshape(-1)[:] = np.concatenate([[0], np.cumsum(flat[:-1])])

    gidx_all, meta_all, dmeta_all = [], [], []
    for c in range(NCORES):
        s_c, p_c, w_c, key = per_core[c]
        g_c = key // NBUK
        b_c = key % NBUK
        cnt = counts[c].reshape(-1)
        starts = np.concatenate([[0], np.cumsum(cnt)[:-1]])
        rank = np.arange(len(s_c)) - starts[key]
        pos = (buk_tile_off[g_c, b_c] * P + rank).astype(np.int64)
        idx16 = [np.zeros(int(buk_len[bb]), dtype=np.int16) for bb in range(NBUK)]
        for bb in range(NBUK):
            mb = b_c == bb
            idx16[bb][pos[mb]] = (s_c[mb] - bb * WIN).astype(np.int16)
        # wrapped layout [128, len/16]: idx i -> (i%16, i//16), replicated 8x
        idx_wrapped = [
            np.ascontiguousarray(np.tile(a.reshape(-1, 16).T, (8, 1)))
            for a in idx16
        ]

        meta = np.zeros((P, NT * 2), dtype=np.float32)
        col = proc_off[g_c, b_c] + (rank >> 7)
        pp = rank & 127
        meta[pp, 2 * col] = p_c.astype(np.float32)
        meta[pp, 2 * col + 1] = w_c

        dmeta = np.zeros((P, NG), dtype=np.float32)
        nm = node_of[c].reshape(NG, P)
        valid = nm >= 0
        dmeta.T[valid] = selfw[nm[valid]]
        gidx_all.append(idx_wrapped)
        meta_all.append(meta)
        dmeta_all.append(dmeta)

    layout = {
        "tiles_gb": tiles_gb,
        "buk_tile_off": buk_tile_off,
        "buk_len": buk_len,
        "proc_off": proc_off,
        "NT": NT,
    }
    return gidx_all, meta_all, dmeta_all, node_of, layout


def _build_program(layout):
    tiles_gb = layout["tiles_gb"]
    buk_tile_off = layout["buk_tile_off"]
    buk_len = layout["buk_len"]
    proc_off = layout["proc_off"]
    NT = layout["NT"]

    nc = bacc.Bacc(
        "TRN2", target_bir_lowering=False, debug=False, num_devices=NCORES,
        num_swdge_queues=NQUEUES,
    )

    x16_d = nc.dram_tensor("x16", [N_NODES, H], BF16, kind="ExternalInput")
    idx_d = [
        nc.dram_tensor(f"idx{b}", [P, int(buk_len[b]) // 16], I16,
                       kind="ExternalInput")
        for b in range(NBUK)
    ]
    meta_d = nc.dram_tensor("meta", [P, NT * 2], F32, kind="ExternalInput")
    dmeta_d = nc.dram_tensor("dmeta", [P, NG], F32, kind="ExternalInput")
    xown_d = nc.dram_tensor("xown", [P, NG * H], F32, kind="ExternalInput")
    iota_d = nc.dram_tensor("iota", [P, P], BF16, kind="ExternalInput")
    iota32_d = nc.dram_tensor("iota32", [P, P], F32, kind="ExternalInput")
    ramp_d = nc.dram_tensor("ramp", [P, 1], F32, kind="ExternalInput")
    ident_d = nc.dram_tensor("ident", [P, P], F32, kind="ExternalInput")
    wg_d = nc.dram_tensor("wg", [H, H], F32, kind="ExternalInput")
    wl_d = nc.dram_tensor("wl", [H, H], F32, kind="ExternalInput")
    bg_d = nc.dram_tensor("bg", [H, 1], F32, kind="ExternalInput")
    bl_d = nc.dram_tensor("bl", [H, 1], F32, kind="ExternalInput")
    out_d = nc.dram_tensor("out", [NPAD, H], F32, kind="ExternalOutput")

    spans = [(g0, min(g0 + SPAN, NG)) for g0 in range(0, NG, SPAN)]
    max_span_tiles = max(int(tiles_gb[g0:g1].sum()) for g0, g1 in spans)

    with tile.TileContext(nc) as tc, ExitStack() as ctx:
        constp = ctx.enter_context(tc.tile_pool(name="const", bufs=1))
        gatherp = ctx.enter_context(tc.tile_pool(name="gather", bufs=2))
        xop = ctx.enter_context(tc.tile_pool(name="xop", bufs=2))
        swp = ctx.enter_context(tc.tile_pool(name="sw", bufs=SW_BUFS))
        workp = ctx.enter_context(tc.tile_pool(name="work", bufs=3))
        aggp = ctx.enter_context(
            tc.tile_pool(name="agg", bufs=PSUM_BUFS, space="PSUM")
        )
        chainp = ctx.enter_context(
            tc.tile_pool(name="chain", bufs=PSUM_BUFS, space="PSUM")
        )

        idx_s = [
            constp.tile([P, int(buk_len[b]) // 16], I16, tag=f"idx{b}",
                        name=f"idx{b}_s")
            for b in range(NBUK)
        ]
        meta_s = constp.tile([P, NT * 2], F32, tag="meta")
        dmeta_s = constp.tile([P, NG], F32, tag="dmeta")
        iota_s = constp.tile([P, P], BF16, tag="iota")
        iota32_s = constp.tile([P, P], F32, tag="iota32")
        ramp_s = constp.tile([P, 1], F32, tag="ramp")
        ident_s = constp.tile([P, P], F32, tag="ident")
        wg_s = constp.tile([H, H], F32, tag="wg")
        wl_s = constp.tile([H, H], F32, tag="wl")
        bg_s = constp.tile([H, 1], F32, tag="bg")
        bl_s = constp.tile([H, 1], F32, tag="bl")
        for sb, dr in [
            (meta_s, meta_d), (dmeta_s, dmeta_d), (iota_s, iota_d),
            (iota32_s, iota32_d), (ramp_s, ramp_d), (ident_s, ident_d),
            (wg_s, wg_d), (wl_s, wl_d), (bg_s, bg_d), (bl_s, bl_d),
        ] + [(idx_s[b], idx_d[b]) for b in range(NBUK)]:
            nc.sync.dma_start(sb[:], dr[:, :])

        for rep in range(REPEAT):
          for si, (g0, g1) in enumerate(spans):
            # per-span gather: one dma_gather per bucket into regions of gb
            gb = gatherp.tile([P, max_span_tiles, H], BF16, tag="gb")
            roff = 0
            span_reg_off = {}  # bucket -> region tile offset in gb
            for b in range(NBUK):
                sb_tiles = int(tiles_gb[g0:g1, b].sum())
                span_reg_off[b] = roff
                n_idx = sb_tiles * P
                c16 = int(buk_tile_off[g0, b]) * P // 16
                if not COMPUTE_ONLY:
                    nc.gpsimd.dma_gather(
                        gb[:, roff : roff + sb_tiles, :],
                        x16_d[b * WIN : min((b + 1) * WIN, N_NODES), :],
                        idx_s[b][:, c16 : c16 + n_idx // 16],
                        n_idx,
                        n_idx,
                        H,
                        single_packet=False,
                        queue_num=(si * NBUK + b) % NQUEUES,
                    )
                roff += sb_tiles
            if COMPUTE_ONLY:
                nc.vector.memset(gb[:, 0:1, :], 0.0)
            xo = xop.tile([P, (g1 - g0) * H], F32, tag="xo")
            nc.sync.dma_start(xo[:], xown_d[:, g0 * H : g1 * H])

            for g in range(g0, g1):
                if GATHER_ONLY:
                    continue
                agg_ps = aggp.tile([H, P], F32, tag="agg")
                ntg = int(tiles_gb[g].sum())
                ti = 0
                for b in range(NBUK):
                    reg = span_reg_off[b] + int(tiles_gb[g0:g, b].sum())
                    for t in range(int(tiles_gb[g, b])):
                        col = int(proc_off[g, b]) + t
                        lt = reg + t
                        sw = swp.tile([P, P], BF16, tag="sw")
                        nc.vector.tensor_scalar(
                            sw[:],
                            iota_s[:],
                            meta_s[:, 2 * col : 2 * col + 1],
                            meta_s[:, 2 * col + 1 : 2 * col + 2],
                            op0=mybir.AluOpType.is_equal,
                            op1=mybir.AluOpType.mult,
                        )
                        nc.tensor.matmul(
                            agg_ps[:],
                            lhsT=gb[:, lt, :],
                            rhs=sw[:],
                            start=(ti == 0),
                            stop=False,
                        )
                        ti += 1
                # self-loop term: aggT += xo_g^T @ diag(1/deg)
                dmat = swp.tile([P, P], F32, tag="dmat")
                nc.vector.tensor_scalar(
                    dmat[:],
                    iota32_s[:],
                    ramp_s[:, 0:1],
                    dmeta_s[:, g : g + 1],
                    op0=mybir.AluOpType.is_equal,
                    op1=mybir.AluOpType.mult,
                )
                nc.tensor.matmul(
                    agg_ps[:],
                    lhsT=xo[:, (g - g0) * H : (g - g0 + 1) * H],
                    rhs=dmat[:],
                    start=False,
                    stop=True,
                )
                # fp32 chain, transposed orientation: [h x d]
                aggT = workp.tile([H, P], F32, tag="aggT")
                nc.scalar.copy(aggT[:], agg_ps[:])
                h1_ps = chainp.tile([H, P], F32, tag="h1ps")
                nc.tensor.matmul(h1_ps[:], lhsT=wg_s[:], rhs=aggT[:],
                                 start=True, stop=True)
                h1 = workp.tile([H, P], F32, tag="h1")
                nc.scalar.activation(
                    h1[:], h1_ps[:], mybir.ActivationFunctionType.Relu,
                    bias=bg_s[:, 0:1], scale=1.0,
                )
                h2_ps = chainp.tile([H, P], F32, tag="h2ps")
                nc.tensor.matmul(h2_ps[:], lhsT=wl_s[:], rhs=h1[:],
                                 start=True, stop=True)
                h2 = workp.tile([H, P], F32, tag="h2")
                nc.scalar.activation(
                    h2[:], h2_ps[:], mybir.ActivationFunctionType.Relu,
                    bias=bl_s[:, 0:1], scale=1.0,
                )
                ht_ps = chainp.tile([P, H], F32, tag="htps")
                nc.tensor.transpose(ht_ps[:], h2[:], ident_s[:])
                outt = workp.tile([P, H], F32, tag="outt")
                nc.vector.tensor_tensor(
                    out=outt[:],
                    in0=ht_ps[:],
                    in1=xo[:, (g - g0) * H : (g - g0 + 1) * H],
                    op=mybir.AluOpType.add,
                )
                nc.sync.dma_start(out_d[g * P : (g + 1) * P, :], outt[:])

    nc.compile()
    return nc


def kernel(x, edge_index, W_gcn, b_gcn, W_lin, b_lin):
    x = np.asarray(x, dtype=np.float32)
    edge_index = np.asarray(edge_index)
    W_gcn = np.asarray(W_gcn, dtype=np.float32)
    b_gcn = np.asarray(b_gcn, dtype=np.float32)
    W_lin = np.asarray(W_lin, dtype=np.float32)
    b_lin = np.asarray(b_lin, dtype=np.float32)

    gidx_all, meta_all, dmeta_all, node_of, layout = _preprocess(x, edge_index)
    nc = _build_program(layout)

    x16 = x.astype(ml_dtypes.bfloat16)
    iota = np.tile(np.arange(P, dtype=np.float32), (P, 1))
    ident = np.eye(P, dtype=np.float32)
    ramp = np.arange(P, dtype=np.float32).reshape(P, 1)
    bg = b_gcn.reshape(H, 1)
    bl = b_lin.reshape(H, 1)

    in_maps = []
    for c in range(NCORES):
        nm = node_of[c]
        xo = np.zeros((NPAD, H), dtype=np.float32)
        valid = nm >= 0
        xo[valid] = x[nm[valid]]
        # [npad, h] -> [p, g*h] so each group's rows sit on partitions
        xo = np.ascontiguousarray(
            xo.reshape(NG, P, H).transpose(1, 0, 2).reshape(P, NG * H)
        )
        m = {
            "x16": x16,
            "meta": meta_all[c],
            "dmeta": dmeta_all[c],
            "xown": xo,
            "iota": iota.astype(ml_dtypes.bfloat16),
            "iota32": iota,
            "ramp": ramp,
            "ident": ident,
            "wg": W_gcn,
            "wl": W_lin,
            "bg": bg,
            "bl": bl,
        }
        for b in range(NBUK):
            m[f"idx{b}"] = gidx_all[c][b]
        in_maps.append(m)

    global LAST_RESULT, LAST_NC, LAST_IN_MAPS
    LAST_NC = nc
    LAST_IN_MAPS = in_maps
    res = run_bass_kernel_spmd(
        nc, in_maps, core_ids=list(range(NCORES)), trace=TRACE
    )
    LAST_RESULT = res
    out = np.empty((N_NODES, H), dtype=np.float32)
    for c in range(NCORES):
        nm = node_of[c]
        valid = nm >= 0
        out[nm[valid]] = res.results[c]["out"][valid]
    return out


if __name__ == "__main__":
    rng = np.random.default_rng(0)
    x = rng.standard_normal((N_NODES, H), dtype=np.float32)
    ei = rng.integers(0, N_NODES, size=(2, N_EDGES)).astype(np.int32)
    s = 1.0 / np.sqrt(H)
    W1 = rng.uniform(-s, s, (H, H)).astype(np.float32)
    b1 = rng.uniform(-s, s, H).astype(np.float32)
    W2 = rng.uniform(-s, s, (H, H)).astype(np.float32)
    b2 = rng.uniform(-s, s, H).astype(np.float32)
    out = kernel(x=x, edge_index=ei, W_gcn=W1, b_gcn=b1, W_lin=W2, b_lin=b2)
    print(out.shape, out.dtype)


# revision 20
# speedup vs baseline: 1.1814x; 1.1814x over previous
"""GCN residual block on 8 Trainium2 NeuronCores.

y = relu(gcn_conv(x)) -> relu(@W_lin + b_lin) -> + x

Strategy (memory-bound regime):
  - Nodes assigned to 8 cores x 98 groups of 128 by round-robin dealing in
    descending in-degree order, which balances edge counts per (group,
    bucket) cell across cores (the SPMD program sizes every cell at the
    max over cores, so balance directly cuts gather padding).
  - Real edges partitioned by dst core, grouped by dst group, bucketed by
    src window (6 windows of 16768 rows so indices fit int16). Gathered
    x[src] rows (bf16, 256B) via gpsimd dma_gather per (span-of-groups,
    bucket), spread across 4 SWDGE queues for concurrent descriptor
    drain (the gather is per-descriptor-rate-bound, not bandwidth-bound).
  - Scatter-add becomes PE matmuls: per 128-edge tile a selection matrix
    S[e, d] = (dst_slot[e] == d) * norm[e] (graph-only data, precomputed
    on host) is streamed from HBM per span, then aggT += G_t^T @ S_t
    accumulated in PSUM per group. Streaming instead of building S on DVE
    removes ~1800 DVE instructions and their per-tile cross-engine syncs.
  - Self-loops never touch DMA: their contribution is one fp32 matmul per
    group, aggT += xo_g^T @ D_g with D_g = diag(1/deg) built on DVE.
  - Per-group fp32 chain in transposed orientation: W^T @ aggT -> relu+bias
    (bias is per-partition there) -> W_lin^T @ . -> relu+bias -> PE
    transpose -> + x residual -> DMA out. Host unpermutes rows at the end.
"""

import sys

sys.path.insert(0, "/opt/trn_rl_repo")

import numpy as np
import ml_dtypes
from contextlib import ExitStack

import concourse.bass as bass
import concourse.mybir as mybir
import concourse.tile as tile
from concourse import bacc
from concourse.bass_utils import run_bass_kernel_spmd

N_NODES = 100000
N_EDGES = 1600000
H = 128
NCORES = 8
P = 128
NG = 98  # groups per core
NGRP = NCORES * NG  # 784 global groups
NPAD = NG * P  # padded nodes per core = 12544
NBUK = 6
WIN = 16768  # src window per bucket (int16-addressable)
SPAN = 8  # groups per gather call batch

F32 = mybir.dt.float32
BF16 = mybir.dt.bfloat16
I16 = mybir.dt.int16

TRACE = False  # set True (e.g. from test.py) to capture an NTFF profile
LAST_RESULT = None
LAST_NC = None
LAST_IN_MAPS = None
GATHER_ONLY = False  # debug: skip compute, only gathers
COMPUTE_ONLY = False  # debug: skip gathers, compute on stale SBUF
REPEAT = 1  # debug: repeat the whole body R times for overhead-free timing
NQUEUES = 4  # SWDGE descriptor queues; bucket b uses queue b % NQUEUES
PSUM_BUFS = 2  # buffers for each PSUM pool (agg, chain)
SW_BUFS = 6  # buffers for the DVE selection-matrix pool


def _preprocess(x, edge_index):
    """Host-side graph prep. Returns per-core SBUF-layout arrays plus the
    shared static layout (tiles per (group, bucket)) and the node
    permutation used for sharding."""
    src = np.ascontiguousarray(edge_index[0]).astype(np.int64)
    dst = np.ascontiguousarray(edge_index[1]).astype(np.int64)

    indeg = np.bincount(dst, minlength=N_NODES)
    deg = (indeg + 1).astype(np.float64)  # + self-loop
    dinv = 1.0 / np.sqrt(deg)
    norm = (dinv[src] * dinv[dst]).astype(np.float32)
    selfw = (dinv * dinv).astype(np.float32)

    # balanced node -> (core, group, slot): deal in descending degree order
    order = np.argsort(-indeg, kind="stable")
    ggrp = np.empty(N_NODES, dtype=np.int64)
    slot = np.empty(N_NODES, dtype=np.int64)
    j = np.arange(N_NODES)
    ggrp[order] = j % NGRP
    slot[order] = j // NGRP
    core_of = ggrp // NG
    g_of = ggrp % NG

    # node_of[c, g*P + p] = global node id or -1
    node_of = np.full((NCORES, NPAD), -1, dtype=np.int64)
    node_of[core_of, g_of * P + slot] = np.arange(N_NODES)

    dc = core_of[dst]
    dg = g_of[dst]
    dp = slot[dst]
    b = src // WIN

    per_core = []
    counts = np.zeros((NCORES, NG, NBUK), dtype=np.int64)
    for c in range(NCORES):
        m = dc == c
        s_c = src[m]
        p_c = dp[m]
        g_c = dg[m]
        w_c = norm[m]
        b_c = b[m]
        key = (g_c * NBUK + b_c).astype(np.int64)
        o = np.argsort(key, kind="stable")
        s_c, p_c, w_c, key = s_c[o], p_c[o], w_c[o], key[o]
        counts[c] = np.bincount(key, minlength=NG * NBUK).reshape(NG, NBUK)
        per_core.append((s_c, p_c, w_c, key))

    # shared across cores: tiles per (group, bucket)
    tiles_gb = (counts.max(axis=0) + P - 1) // P  # [NG, NBUK]
    tiles_gb = np.maximum(tiles_gb, 1)
    NT = int(tiles_gb.sum())  # total tile columns per core

    # per-bucket edge-slot layout offsets (tiles), ordered by group
    buk_tile_off = np.zeros((NG, NBUK), dtype=np.int64)
    for bb in range(NBUK):
        buk_tile_off[:, bb] = np.concatenate([[0], np.cumsum(tiles_gb[:-1, bb])])
    buk_len = tiles_gb.sum(axis=0) * P  # slots per bucket stream

    # processing order (meta columns): for g, for b, for t
    proc_off = np.zeros((NG, NBUK), dtype=np.int64)
    flat = tiles_gb.reshape(-1)
    proc_off.reshape(-1)[:] = np.concatenate([[0], np.cumsum(flat[:-1])])

    gidx_all, meta_all, dmeta_all = [], [], []
    for c in range(NCORES):
        s_c, p_c, w_c, key = per_core[c]
        g_c = key // NBUK
        b_c = key % NBUK
        cnt = counts[c].reshape(-1)
        starts = np.concatenate([[0], np.cumsum(cnt)[:-1]])
        rank = np.arange(len(s_c)) - starts[key]
        pos = (buk_tile_off[g_c, b_c] * P + rank).astype(np.int64)
        idx16 = [np.zeros(int(buk_len[bb]), dtype=np.int16) for bb in range(NBUK)]
        for bb in range(NBUK):
            mb = b_c == bb
            idx16[bb][pos[mb]] = (s_c[mb] - bb * WIN).astype(np.int16)
        # wrapped layout [128, len/16]: idx i -> (i%16, i//16), replicated 8x
        idx_wrapped = [
            np.ascontiguousarray(np.tile(a.reshape(-1, 16).T, (8, 1)))
            for a in idx16
        ]

        # precomputed selection matrices, streamed per span on device:
        # tile t (proc order), sw[e, d] = (dst_slot[e] == d) * norm[e]
        sw = np.zeros((P, NT * P), dtype=ml_dtypes.bfloat16)
        col = proc_off[g_c, b_c] + (rank >> 7)
        pp = rank & 127
        sw[pp, col * P + p_c] = w_c

        dmeta = np.zeros((P, NG), dtype=np.float32)
        nm = node_of[c].reshape(NG, P)
        valid = nm >= 0
        dmeta.T[valid] = selfw[nm[valid]]
        gidx_all.append(idx_wrapped)
        meta_all.append(sw)
        dmeta_all.append(dmeta)

    layout = {
        "tiles_gb": tiles_gb,
        "buk_tile_off": buk_tile_off,
        "buk_len": buk_len,
        "proc_off": proc_off,
        "NT": NT,
    }
    return gidx_all, meta_all, dmeta_all, node_of, layout


def _build_program(layout):
    tiles_gb = layout["tiles_gb"]
    buk_tile_off = layout["buk_tile_off"]
    buk_len = layout["buk_len"]
    proc_off = layout["proc_off"]
    NT = layout["NT"]

    nc = bacc.Bacc(
        "TRN2", target_bir_lowering=False, debug=False, num_devices=NCORES,
        num_swdge_queues=NQUEUES,
    )

    x16_d = nc.dram_tensor("x16", [N_NODES, H], BF16, kind="ExternalInput")
    idx_d = [
        nc.dram_tensor(f"idx{b}", [P, int(buk_len[b]) // 16], I16,
                       kind="ExternalInput")
        for b in range(NBUK)
    ]
    sw_d = nc.dram_tensor("sw", [P, NT * P], BF16, kind="ExternalInput")
    dmeta_d = nc.dram_tensor("dmeta", [P, NG], F32, kind="ExternalInput")
    xown_d = nc.dram_tensor("xown", [P, NG * H], F32, kind="ExternalInput")
    iota32_d = nc.dram_tensor("iota32", [P, P], F32, kind="ExternalInput")
    ramp_d = nc.dram_tensor("ramp", [P, 1], F32, kind="ExternalInput")
    ident_d = nc.dram_tensor("ident", [P, P], F32, kind="ExternalInput")
    wg_d = nc.dram_tensor("wg", [H, H], F32, kind="ExternalInput")
    wl_d = nc.dram_tensor("wl", [H, H], F32, kind="ExternalInput")
    bg_d = nc.dram_tensor("bg", [H, 1], F32, kind="ExternalInput")
    bl_d = nc.dram_tensor("bl", [H, 1], F32, kind="ExternalInput")
    out_d = nc.dram_tensor("out", [NPAD, H], F32, kind="ExternalOutput")

    spans = [(g0, min(g0 + SPAN, NG)) for g0 in range(0, NG, SPAN)]
    max_span_tiles = max(int(tiles_gb[g0:g1].sum()) for g0, g1 in spans)

    with tile.TileContext(nc) as tc, ExitStack() as ctx:
        constp = ctx.enter_context(tc.tile_pool(name="const", bufs=1))
        gatherp = ctx.enter_context(tc.tile_pool(name="gather", bufs=2))
        swsp = ctx.enter_context(tc.tile_pool(name="sws", bufs=2))
        xop = ctx.enter_context(tc.tile_pool(name="xop", bufs=2))
        swp = ctx.enter_context(tc.tile_pool(name="sw", bufs=SW_BUFS))
        workp = ctx.enter_context(tc.tile_pool(name="work", bufs=3))
        aggp = ctx.enter_context(
            tc.tile_pool(name="agg", bufs=PSUM_BUFS, space="PSUM")
        )
        chainp = ctx.enter_context(
            tc.tile_pool(name="chain", bufs=PSUM_BUFS, space="PSUM")
        )

        idx_s = [
            constp.tile([P, int(buk_len[b]) // 16], I16, tag=f"idx{b}",
                        name=f"idx{b}_s")
            for b in range(NBUK)
        ]
        dmeta_s = constp.tile([P, NG], F32, tag="dmeta")
        iota32_s = constp.tile([P, P], F32, tag="iota32")
        ramp_s = constp.tile([P, 1], F32, tag="ramp")
        ident_s = constp.tile([P, P], F32, tag="ident")
        wg_s = constp.tile([H, H], F32, tag="wg")
        wl_s = constp.tile([H, H], F32, tag="wl")
        bg_s = constp.tile([H, 1], F32, tag="bg")
        bl_s = constp.tile([H, 1], F32, tag="bl")
        for sb, dr in [
            (dmeta_s, dmeta_d),
            (iota32_s, iota32_d), (ramp_s, ramp_d), (ident_s, ident_d),
            (wg_s, wg_d), (wl_s, wl_d), (bg_s, bg_d), (bl_s, bl_d),
        ] + [(idx_s[b], idx_d[b]) for b in range(NBUK)]:
            nc.sync.dma_start(sb[:], dr[:, :])

        for rep in range(REPEAT):
          for si, (g0, g1) in enumerate(spans):
            # per-span gather: one dma_gather per bucket into regions of gb
            gb = gatherp.tile([P, max_span_tiles, H], BF16, tag="gb")
            roff = 0
            span_reg_off = {}  # bucket -> region tile offset in gb
            for b in range(NBUK):
                sb_tiles = int(tiles_gb[g0:g1, b].sum())
                span_reg_off[b] = roff
                n_idx = sb_tiles * P
                c16 = int(buk_tile_off[g0, b]) * P // 16
                if not COMPUTE_ONLY:
                    nc.gpsimd.dma_gather(
                        gb[:, roff : roff + sb_tiles, :],
                        x16_d[b * WIN : min((b + 1) * WIN, N_NODES), :],
                        idx_s[b][:, c16 : c16 + n_idx // 16],
                        n_idx,
                        n_idx,
                        H,
                        single_packet=False,
                        queue_num=(si * NBUK + b) % NQUEUES,
                    )
                roff += sb_tiles
            if COMPUTE_ONLY:
                nc.vector.memset(gb[:, 0:1, :], 0.0)
            span_t0 = int(proc_off[g0, 0])  # tiles before this span
            span_nt = int(tiles_gb[g0:g1].sum())
            sws = swsp.tile([P, max_span_tiles * P], BF16, tag="sws")
            nc.sync.dma_start(
                sws[:, : span_nt * P],
                sw_d[:, span_t0 * P : (span_t0 + span_nt) * P],
            )
            xo = xop.tile([P, (g1 - g0) * H], F32, tag="xo")
            nc.sync.dma_start(xo[:], xown_d[:, g0 * H : g1 * H])

            for g in range(g0, g1):
                if GATHER_ONLY:
                    continue
                agg_ps = aggp.tile([H, P], F32, tag="agg")
                ntg = int(tiles_gb[g].sum())
                ti = 0
                for b in range(NBUK):
                    reg = span_reg_off[b] + int(tiles_gb[g0:g, b].sum())
                    for t in range(int(tiles_gb[g, b])):
                        sc = int(proc_off[g, b]) + t - span_t0
                        lt = reg + t
                        nc.tensor.matmul(
                            agg_ps[:],
                            lhsT=gb[:, lt, :],
                            rhs=sws[:, sc * P : (sc + 1) * P],
                            start=(ti == 0),
                            stop=False,
                        )
                        ti += 1
                # self-loop term: aggT += xo_g^T @ diag(1/deg)
                dmat = swp.tile([P, P], F32, tag="dmat")
                nc.vector.tensor_scalar(
                    dmat[:],
                    iota32_s[:],
                    ramp_s[:, 0:1],
                    dmeta_s[:, g : g + 1],
                    op0=mybir.AluOpType.is_equal,
                    op1=mybir.AluOpType.mult,
                )
                nc.tensor.matmul(
                    agg_ps[:],
                    lhsT=xo[:, (g - g0) * H : (g - g0 + 1) * H],
                    rhs=dmat[:],
                    start=False,
                    stop=True,
                )
                # fp32 chain, transposed orientation: [h x d]
                aggT = workp.tile([H, P], F32, tag="aggT")
                nc.scalar.copy(aggT[:], agg_ps[:])
                h1_ps = chainp.tile([H, P], F32, tag="h1ps")
                nc.tensor.matmul(h1_ps[:], lhsT=wg_s[:], rhs=aggT[:],
                                 start=True, stop=True)
                h1 = workp.tile([H, P], F32, tag="h1")
                nc.scalar.activation(
                    h1[:], h1_ps[:], mybir.ActivationFunctionType.Relu,
                    bias=bg_s[:, 0:1], scale=1.0,
                )
                h2_ps = chainp.tile([H, P], F32, tag="h2ps")
                nc.tensor.matmul(h2_ps[:], lhsT=wl_s[:], rhs=h1[:],
                                 start=True, stop=True)
                h2 = workp.tile([H, P], F32, tag="h2")
                nc.scalar.activation(
                    h2[:], h2_ps[:], mybir.ActivationFunctionType.Relu,
                    bias=bl_s[:, 0:1], scale=1.0,
                )
                ht_ps = chainp.tile([P, H], F32, tag="htps")
                nc.tensor.transpose(ht_ps[:], h2[:], ident_s[:])
                outt = workp.tile([P, H], F32, tag="outt")
                nc.vector.tensor_tensor(
                    out=outt[:],
                    in0=ht_ps[:],
                    in1=xo[:, (g - g0) * H : (g - g0 + 1) * H],
                    op=mybir.AluOpType.add,
                )
                nc.sync.dma_start(out_d[g * P : (g + 1) * P, :], outt[:])

    nc.compile()
    return nc


def kernel(x, edge_index, W_gcn, b_gcn, W_lin, b_lin):
    x = np.asarray(x, dtype=np.float32)
    edge_index = np.asarray(edge_index)
    W_gcn = np.asarray(W_gcn, dtype=np.float32)
    b_gcn = np.asarray(b_gcn, dtype=np.float32)
    W_lin = np.asarray(W_lin, dtype=np.float32)
    b_lin = np.asarray(b_lin, dtype=np.float32)

    gidx_all, meta_all, dmeta_all, node_of, layout = _preprocess(x, edge_index)
    nc = _build_program(layout)

    x16 = x.astype(ml_dtypes.bfloat16)
    iota32 = np.tile(np.arange(P, dtype=np.float32), (P, 1))
    ident = np.eye(P, dtype=np.float32)
    ramp = np.arange(P, dtype=np.float32).reshape(P, 1)
    bg = b_gcn.reshape(H, 1)
    bl = b_lin.reshape(H, 1)

    in_maps = []
    for c in range(NCORES):
        nm = node_of[c]
        xo = np.zeros((NPAD, H), dtype=np.float32)
        valid = nm >= 0
        xo[valid] = x[nm[valid]]
        # [npad, h] -> [p, g*h] so each group's rows sit on partitions
        xo = np.ascontiguousarray(
            xo.reshape(NG, P, H).transpose(1, 0, 2).reshape(P, NG * H)
        )
        m = {
            "x16": x16,
            "sw": meta_all[c],
            "dmeta": dmeta_all[c],
            "xown": xo,
            "iota32": iota32,
            "ramp": ramp,
            "ident": ident,
            "wg": W_gcn,
            "wl": W_lin,
            "bg": bg,
            "bl": bl,
        }
        for b in range(NBUK):
            m[f"idx{b}"] = gidx_all[c][b]
        in_maps.append(m)

    global LAST_RESULT, LAST_NC, LAST_IN_MAPS
    LAST_NC = nc
    LAST_IN_MAPS = in_maps
    res = run_bass_kernel_spmd(
        nc, in_maps, core_ids=list(range(NCORES)), trace=TRACE
    )
    LAST_RESULT = res
    out = np.empty((N_NODES, H), dtype=np.float32)
    for c in range(NCORES):
        nm = node_of[c]
        valid = nm >= 0
        out[nm[valid]] = res.results[c]["out"][valid]
    return out


if __name__ == "__main__":
    rng = np.random.default_rng(0)
    x = rng.standard_normal((N_NODES, H), dtype=np.float32)
    ei = rng.integers(0, N_NODES, size=(2, N_EDGES)).astype(np.int32)
    s = 1.0 / np.sqrt(H)
    W1 = rng.uniform(-s, s, (H, H)).astype(np.float32)
    b1 = rng.uniform(-s, s, H).astype(np.float32)
    W2 = rng.uniform(-s, s, (H, H)).astype(np.float32)
    b2 = rng.uniform(-s, s, H).astype(np.float32)
    out = kernel(x=x, edge_index=ei, W_gcn=W1, b_gcn=b1, W_lin=W2, b_lin=b2)
    print(out.shape, out.dtype)


# revision 25
# speedup vs baseline: 1.2699x; 1.0749x over previous
"""GCN residual block on 8 Trainium2 NeuronCores.

y = relu(gcn_conv(x)) -> relu(@W_lin + b_lin) -> + x

Strategy (memory-bound regime):
  - Nodes assigned to 8 cores x 98 groups of 128 by round-robin dealing in
    descending in-degree order, which balances edge counts per (group,
    bucket) cell across cores (the SPMD program sizes every cell at the
    max over cores, so balance directly cuts gather padding).
  - Real edges partitioned by dst core, grouped by dst group, bucketed by
    src window (6 windows of 16768 rows so indices fit int16). Gathered
    x[src] rows (bf16, 256B) via gpsimd dma_gather per (span-of-groups,
    bucket), spread across 4 SWDGE queues for concurrent descriptor
    drain (the gather is per-descriptor-rate-bound, not bandwidth-bound).
  - Scatter-add becomes PE matmuls: per 128-edge tile a selection matrix
    S[e, d] = (dst_slot[e] == d) * norm[e] (graph-only data, precomputed
    on host) is streamed from HBM per span, then aggT += G_t^T @ S_t
    accumulated in PSUM per group. Streaming instead of building S on DVE
    removes ~1800 DVE instructions and their per-tile cross-engine syncs.
  - Self-loops never touch DMA: their contribution is one fp32 matmul per
    group, aggT += xo_g^T @ D_g with D_g = diag(1/deg) built on DVE.
  - Per-group fp32 chain in transposed orientation: W^T @ aggT -> relu+bias
    (bias is per-partition there) -> W_lin^T @ . -> relu+bias -> PE
    transpose -> + x residual -> DMA out. Host unpermutes rows at the end.
"""

import sys

sys.path.insert(0, "/opt/trn_rl_repo")

import numpy as np
import ml_dtypes
from contextlib import ExitStack

import concourse.bass as bass
import concourse.mybir as mybir
import concourse.tile as tile
from concourse import bacc
from concourse.bass_utils import run_bass_kernel_spmd

N_NODES = 100000
N_EDGES = 1600000
H = 128
NCORES = 8
P = 128
NG = 98  # groups per core
NGRP = NCORES * NG  # 784 global groups
NPAD = NG * P  # padded nodes per core = 12544
NBUK = 6
WIN = 16768  # src window per bucket (int16-addressable)
SPAN = 8  # groups per gather call batch

F32 = mybir.dt.float32
BF16 = mybir.dt.bfloat16
I16 = mybir.dt.int16

TRACE = False  # set True (e.g. from test.py) to capture an NTFF profile
LAST_RESULT = None
LAST_NC = None
LAST_IN_MAPS = None
GATHER_ONLY = False  # debug: skip compute, only gathers
COMPUTE_ONLY = False  # debug: skip gathers, compute on stale SBUF
REPEAT = 1  # debug: repeat the whole body R times for overhead-free timing
NQUEUES = 4  # SWDGE descriptor queues; bucket b uses queue b % NQUEUES
PSUM_BUFS = 2  # buffers for each PSUM pool (agg, chain)
SW_BUFS = 6  # buffers for the DVE selection-matrix pool


def _preprocess(x, edge_index):
    """Host-side graph prep. Returns per-core SBUF-layout arrays plus the
    shared static layout (tiles per (group, bucket)) and the node
    permutation used for sharding."""
    src = np.ascontiguousarray(edge_index[0]).astype(np.int64)
    dst = np.ascontiguousarray(edge_index[1]).astype(np.int64)

    indeg = np.bincount(dst, minlength=N_NODES)
    deg = (indeg + 1).astype(np.float64)  # + self-loop
    dinv = 1.0 / np.sqrt(deg)
    norm = (dinv[src] * dinv[dst]).astype(np.float32)
    selfw = (dinv * dinv).astype(np.float32)

    # balanced node -> (core, group, slot): deal in descending degree order
    order = np.argsort(-indeg, kind="stable")
    ggrp = np.empty(N_NODES, dtype=np.int64)
    slot = np.empty(N_NODES, dtype=np.int64)
    j = np.arange(N_NODES)
    ggrp[order] = j % NGRP
    slot[order] = j // NGRP
    core_of = ggrp // NG
    g_of = ggrp % NG

    # node_of[c, g*P + p] = global node id or -1
    node_of = np.full((NCORES, NPAD), -1, dtype=np.int64)
    node_of[core_of, g_of * P + slot] = np.arange(N_NODES)

    dc = core_of[dst]
    dg = g_of[dst]
    dp = slot[dst]
    b = src // WIN

    per_core = []
    counts = np.zeros((NCORES, NG, NBUK), dtype=np.int64)
    for c in range(NCORES):
        m = dc == c
        s_c = src[m]
        p_c = dp[m]
        g_c = dg[m]
        w_c = norm[m]
        b_c = b[m]
        key = (g_c * NBUK + b_c).astype(np.int64)
        o = np.argsort(key, kind="stable")
        s_c, p_c, w_c, key = s_c[o], p_c[o], w_c[o], key[o]
        counts[c] = np.bincount(key, minlength=NG * NBUK).reshape(NG, NBUK)
        per_core.append((s_c, p_c, w_c, key))

    # shared across cores: tiles per (group, bucket)
    tiles_gb = (counts.max(axis=0) + P - 1) // P  # [NG, NBUK]
    tiles_gb = np.maximum(tiles_gb, 1)
    NT = int(tiles_gb.sum())  # total tile columns per core

    # per-bucket edge-slot layout offsets (tiles), ordered by group
    buk_tile_off = np.zeros((NG, NBUK), dtype=np.int64)
    for bb in range(NBUK):
        buk_tile_off[:, bb] = np.concatenate([[0], np.cumsum(tiles_gb[:-1, bb])])
    buk_len = tiles_gb.sum(axis=0) * P  # slots per bucket stream

    # processing order (meta columns): for g, for b, for t
    proc_off = np.zeros((NG, NBUK), dtype=np.int64)
    flat = tiles_gb.reshape(-1)
    proc_off.reshape(-1)[:] = np.concatenate([[0], np.cumsum(flat[:-1])])

    gidx_all, meta_all, dmeta_all = [], [], []
    for c in range(NCORES):
        s_c, p_c, w_c, key = per_core[c]
        g_c = key // NBUK
        b_c = key % NBUK
        cnt = counts[c].reshape(-1)
        starts = np.concatenate([[0], np.cumsum(cnt)[:-1]])
        rank = np.arange(len(s_c)) - starts[key]
        pos = (buk_tile_off[g_c, b_c] * P + rank).astype(np.int64)
        idx16 = [np.zeros(int(buk_len[bb]), dtype=np.int16) for bb in range(NBUK)]
        for bb in range(NBUK):
            mb = b_c == bb
            idx16[bb][pos[mb]] = (s_c[mb] - bb * WIN).astype(np.int16)
        # wrapped layout [128, len/16]: idx i -> (i%16, i//16), replicated 8x
        idx_wrapped = [
            np.ascontiguousarray(np.tile(a.reshape(-1, 16).T, (8, 1)))
            for a in idx16
        ]

        # precomputed selection matrices, streamed per span on device:
        # tile t (proc order), sw[e, d] = (dst_slot[e] == d) * norm[e]
        sw = np.zeros((P, NT * P), dtype=ml_dtypes.bfloat16)
        col = proc_off[g_c, b_c] + (rank >> 7)
        pp = rank & 127
        sw[pp, col * P + p_c] = w_c

        dmeta = np.zeros((P, NG), dtype=np.float32)
        nm = node_of[c].reshape(NG, P)
        valid = nm >= 0
        dmeta.T[valid] = selfw[nm[valid]]
        gidx_all.append(idx_wrapped)
        meta_all.append(sw)
        dmeta_all.append(dmeta)

    layout = {
        "tiles_gb": tiles_gb,
        "buk_tile_off": buk_tile_off,
        "buk_len": buk_len,
        "proc_off": proc_off,
        "NT": NT,
    }
    return gidx_all, meta_all, dmeta_all, node_of, layout


def _build_program(layout):
    tiles_gb = layout["tiles_gb"]
    buk_tile_off = layout["buk_tile_off"]
    buk_len = layout["buk_len"]
    proc_off = layout["proc_off"]
    NT = layout["NT"]

    nc = bacc.Bacc(
        "TRN2", target_bir_lowering=False, debug=False, num_devices=NCORES,
        num_swdge_queues=NQUEUES,
    )

    x16_d = nc.dram_tensor("x16", [N_NODES, H], BF16, kind="ExternalInput")
    idx_d = [
        nc.dram_tensor(f"idx{b}", [P, int(buk_len[b]) // 16], I16,
                       kind="ExternalInput")
        for b in range(NBUK)
    ]
    sw_d = nc.dram_tensor("sw", [P, NT * P], BF16, kind="ExternalInput")
    dmeta_d = nc.dram_tensor("dmeta", [P, NG], F32, kind="ExternalInput")
    xown_d = nc.dram_tensor("xown", [P, NG * H], F32, kind="ExternalInput")
    iota32_d = nc.dram_tensor("iota32", [P, P], F32, kind="ExternalInput")
    ramp_d = nc.dram_tensor("ramp", [P, 1], F32, kind="ExternalInput")
    ident_d = nc.dram_tensor("ident", [P, P], F32, kind="ExternalInput")
    wg_d = nc.dram_tensor("wg", [H, H], F32, kind="ExternalInput")
    wl_d = nc.dram_tensor("wl", [H, H], F32, kind="ExternalInput")
    bg_d = nc.dram_tensor("bg", [H, 1], F32, kind="ExternalInput")
    bl_d = nc.dram_tensor("bl", [H, 1], F32, kind="ExternalInput")
    # partition-major output: [p, g*H] so each span's store is 128 x 4KB
    # contiguous descriptors instead of 12.5k x 512B; host undoes the wrap
    out_d = nc.dram_tensor("out", [P, NG * H], F32, kind="ExternalOutput")

    spans = [(g0, min(g0 + SPAN, NG)) for g0 in range(0, NG, SPAN)]
    max_span_tiles = max(int(tiles_gb[g0:g1].sum()) for g0, g1 in spans)

    with tile.TileContext(nc) as tc, ExitStack() as ctx:
        constp = ctx.enter_context(tc.tile_pool(name="const", bufs=1))
        gatherp = ctx.enter_context(tc.tile_pool(name="gather", bufs=2))
        swsp = ctx.enter_context(tc.tile_pool(name="sws", bufs=2))
        xop = ctx.enter_context(tc.tile_pool(name="xop", bufs=2))
        swp = ctx.enter_context(tc.tile_pool(name="sw", bufs=SW_BUFS))
        workp = ctx.enter_context(tc.tile_pool(name="work", bufs=3))
        aggp = ctx.enter_context(
            tc.tile_pool(name="agg", bufs=PSUM_BUFS, space="PSUM")
        )
        chainp = ctx.enter_context(
            tc.tile_pool(name="chain", bufs=PSUM_BUFS, space="PSUM")
        )

        idx_s = [
            constp.tile([P, int(buk_len[b]) // 16], I16, tag=f"idx{b}",
                        name=f"idx{b}_s")
            for b in range(NBUK)
        ]
        dmeta_s = constp.tile([P, NG], F32, tag="dmeta")
        iota32_s = constp.tile([P, P], F32, tag="iota32")
        ramp_s = constp.tile([P, 1], F32, tag="ramp")
        ident_s = constp.tile([P, P], F32, tag="ident")
        wg_s = constp.tile([H, H], F32, tag="wg")
        wl_s = constp.tile([H, H], F32, tag="wl")
        bg_s = constp.tile([H, 1], F32, tag="bg")
        bl_s = constp.tile([H, 1], F32, tag="bl")
        for sb, dr in [
            (dmeta_s, dmeta_d),
            (iota32_s, iota32_d), (ramp_s, ramp_d), (ident_s, ident_d),
            (wg_s, wg_d), (wl_s, wl_d), (bg_s, bg_d), (bl_s, bl_d),
        ] + [(idx_s[b], idx_d[b]) for b in range(NBUK)]:
            nc.sync.dma_start(sb[:], dr[:, :])

        for rep in range(REPEAT):
          for si, (g0, g1) in enumerate(spans):
            # per-span gather: one dma_gather per bucket into regions of gb
            gb = gatherp.tile([P, max_span_tiles, H], BF16, tag="gb")
            roff = 0
            span_reg_off = {}  # bucket -> region tile offset in gb
            for b in range(NBUK):
                sb_tiles = int(tiles_gb[g0:g1, b].sum())
                span_reg_off[b] = roff
                n_idx = sb_tiles * P
                c16 = int(buk_tile_off[g0, b]) * P // 16
                if not COMPUTE_ONLY:
                    nc.gpsimd.dma_gather(
                        gb[:, roff : roff + sb_tiles, :],
                        x16_d[b * WIN : min((b + 1) * WIN, N_NODES), :],
                        idx_s[b][:, c16 : c16 + n_idx // 16],
                        n_idx,
                        n_idx,
                        H,
                        single_packet=False,
                        queue_num=(si * NBUK + b) % NQUEUES,
                    )
                roff += sb_tiles
            if COMPUTE_ONLY:
                nc.vector.memset(gb[:, 0:1, :], 0.0)
            span_t0 = int(proc_off[g0, 0])  # tiles before this span
            span_nt = int(tiles_gb[g0:g1].sum())
            sws = swsp.tile([P, max_span_tiles * P], BF16, tag="sws")
            nc.sync.dma_start(
                sws[:, : span_nt * P],
                sw_d[:, span_t0 * P : (span_t0 + span_nt) * P],
            )
            xo = xop.tile([P, (g1 - g0) * H], F32, tag="xo")
            nc.sync.dma_start(xo[:], xown_d[:, g0 * H : g1 * H])
            if not GATHER_ONLY:
                outsp = workp.tile([P, (g1 - g0) * H], F32, tag="outsp")

            for g in range(g0, g1):
                if GATHER_ONLY:
                    continue
                agg_ps = aggp.tile([H, P], F32, tag="agg")
                ntg = int(tiles_gb[g].sum())
                ti = 0
                for b in range(NBUK):
                    reg = span_reg_off[b] + int(tiles_gb[g0:g, b].sum())
                    for t in range(int(tiles_gb[g, b])):
                        sc = int(proc_off[g, b]) + t - span_t0
                        lt = reg + t
                        nc.tensor.matmul(
                            agg_ps[:],
                            lhsT=gb[:, lt, :],
                            rhs=sws[:, sc * P : (sc + 1) * P],
                            start=(ti == 0),
                            stop=False,
                        )
                        ti += 1
                # self-loop term: aggT += xo_g^T @ diag(1/deg)
                dmat = swp.tile([P, P], F32, tag="dmat")
                nc.vector.tensor_scalar(
                    dmat[:],
                    iota32_s[:],
                    ramp_s[:, 0:1],
                    dmeta_s[:, g : g + 1],
                    op0=mybir.AluOpType.is_equal,
                    op1=mybir.AluOpType.mult,
                )
                nc.tensor.matmul(
                    agg_ps[:],
                    lhsT=xo[:, (g - g0) * H : (g - g0 + 1) * H],
                    rhs=dmat[:],
                    start=False,
                    stop=True,
                )
                # fp32 chain, transposed orientation: [h x d]
                aggT = workp.tile([H, P], F32, tag="aggT")
                nc.scalar.copy(aggT[:], agg_ps[:])
                h1_ps = chainp.tile([H, P], F32, tag="h1ps")
                nc.tensor.matmul(h1_ps[:], lhsT=wg_s[:], rhs=aggT[:],
                                 start=True, stop=True)
                h1 = workp.tile([H, P], F32, tag="h1")
                nc.scalar.activation(
                    h1[:], h1_ps[:], mybir.ActivationFunctionType.Relu,
                    bias=bg_s[:, 0:1], scale=1.0,
                )
                h2_ps = chainp.tile([H, P], F32, tag="h2ps")
                nc.tensor.matmul(h2_ps[:], lhsT=wl_s[:], rhs=h1[:],
                                 start=True, stop=True)
                h2 = workp.tile([H, P], F32, tag="h2")
                nc.scalar.activation(
                    h2[:], h2_ps[:], mybir.ActivationFunctionType.Relu,
                    bias=bl_s[:, 0:1], scale=1.0,
                )
                ht_ps = chainp.tile([P, H], F32, tag="htps")
                nc.tensor.transpose(ht_ps[:], h2[:], ident_s[:])
                nc.vector.tensor_tensor(
                    out=outsp[:, (g - g0) * H : (g - g0 + 1) * H],
                    in0=ht_ps[:],
                    in1=xo[:, (g - g0) * H : (g - g0 + 1) * H],
                    op=mybir.AluOpType.add,
                )
            if not GATHER_ONLY:
                nc.sync.dma_start(
                    out_d[:, g0 * H : g1 * H], outsp[:]
                )

    nc.compile()
    return nc


def kernel(x, edge_index, W_gcn, b_gcn, W_lin, b_lin):
    x = np.asarray(x, dtype=np.float32)
    edge_index = np.asarray(edge_index)
    W_gcn = np.asarray(W_gcn, dtype=np.float32)
    b_gcn = np.asarray(b_gcn, dtype=np.float32)
    W_lin = np.asarray(W_lin, dtype=np.float32)
    b_lin = np.asarray(b_lin, dtype=np.float32)

    gidx_all, meta_all, dmeta_all, node_of, layout = _preprocess(x, edge_index)
    nc = _build_program(layout)

    x16 = x.astype(ml_dtypes.bfloat16)
    iota32 = np.tile(np.arange(P, dtype=np.float32), (P, 1))
    ident = np.eye(P, dtype=np.float32)
    ramp = np.arange(P, dtype=np.float32).reshape(P, 1)
    bg = b_gcn.reshape(H, 1)
    bl = b_lin.reshape(H, 1)

    in_maps = []
    for c in range(NCORES):
        nm = node_of[c]
        xo = np.zeros((NPAD, H), dtype=np.float32)
        valid = nm >= 0
        xo[valid] = x[nm[valid]]
        # [npad, h] -> [p, g*h] so each group's rows sit on partitions
        xo = np.ascontiguousarray(
            xo.reshape(NG, P, H).transpose(1, 0, 2).reshape(P, NG * H)
        )
        m = {
            "x16": x16,
            "sw": meta_all[c],
            "dmeta": dmeta_all[c],
            "xown": xo,
            "iota32": iota32,
            "ramp": ramp,
            "ident": ident,
            "wg": W_gcn,
            "wl": W_lin,
            "bg": bg,
            "bl": bl,
        }
        for b in range(NBUK):
            m[f"idx{b}"] = gidx_all[c][b]
        in_maps.append(m)

    global LAST_RESULT, LAST_NC, LAST_IN_MAPS
    LAST_NC = nc
    LAST_IN_MAPS = in_maps
    res = run_bass_kernel_spmd(
        nc, in_maps, core_ids=list(range(NCORES)), trace=TRACE
    )
    LAST_RESULT = res
    out = np.empty((N_NODES, H), dtype=np.float32)
    for c in range(NCORES):
        nm = node_of[c]
        valid = nm >= 0
        # [p, g*H] wrap -> [npad, H]
        oc = (
            res.results[c]["out"]
            .reshape(P, NG, H)
            .transpose(1, 0, 2)
            .reshape(NPAD, H)
        )
        out[nm[valid]] = oc[valid]
    return out


if __name__ == "__main__":
    rng = np.random.default_rng(0)
    x = rng.standard_normal((N_NODES, H), dtype=np.float32)
    ei = rng.integers(0, N_NODES, size=(2, N_EDGES)).astype(np.int32)
    s = 1.0 / np.sqrt(H)
    W1 = rng.uniform(-s, s, (H, H)).astype(np.float32)
    b1 = rng.uniform(-s, s, H).astype(np.float32)
    W2 = rng.uniform(-s, s, (H, H)).astype(np.float32)
    b2 = rng.uniform(-s, s, H).astype(np.float32)
    out = kernel(x=x, edge_index=ei, W_gcn=W1, b_gcn=b1, W_lin=W2, b_lin=b2)
    print(out.shape, out.dtype)
